# revision 1
# baseline (speedup 1.0000x reference)
"""Distributed Trainium2 kernel for nn_Attention_68719477187.

RoPE + causal GQA attention (B=2, S=2048, DIM=2048, 32 q heads / 8 kv heads,
head_dim 64) on 8 NeuronCores: DP=2 over batch x TP=4 over head groups.

Per core (b = core//4, G = core%4): 8 q heads / 2 kv heads of batch b.
  1. qkv.T = w{q,k,v}T.T @ x_b.T (contraction over model dim on partitions)
  2. RoPE applied in transposed layout; head_dim pre-permuted (evens, odds)
     on the host so rotation pairs become contiguous 32-partition blocks.
  3. scores.T tiles (k on partitions, q on free) -> exp (no max subtraction;
     scores are O(5) so fp32 exp is safe) -> causal mask by 0/1 multiply ->
     AV matmul with a ones-column appended to V so the softmax denominator
     falls out of the same matmul.
  4. AllGather attention outputs (bf16, chunked per 512 seq positions and
     pipelined behind later attention chunks) within each batch group of 4
     cores, then each core computes a 512-column slice of wo.

Phases are interleaved per sequence chunk sc: qkv(sc) -> attention(qc=sc)
-> AllGather(sc) -> wo(sc-1), so PE matmul work overlaps the ACT-bound
softmax and the collectives.

Compute in bf16 (fp32 PSUM accumulation), output fp32.
"""

import sys

if "/opt/trn_rl_repo" not in sys.path:
    sys.path.insert(0, "/opt/trn_rl_repo")

import numpy as np
import ml_dtypes

from concourse import bacc, tile, mybir
from concourse.bass_utils import run_bass_kernel_spmd

BF16 = ml_dtypes.bfloat16

S = 2048          # sequence length
D = 2048          # model dim
HD = 64           # head dim
NQL = 8           # local q heads
NKVL = 2          # local kv heads
QC = 512          # q chunk (matmul free dim)
NSC = S // QC     # 4 seq chunks
NKD = D // 128    # 16 contraction tiles
NKT = S // 128    # 16 key tiles
SCALE = HD ** -0.5

_NC = None


def _build(_no_cc=False):
    import os
    # "mm,st,av,pp" pool-depth override, used only for tuning experiments;
    # any malformed/absent value falls back to the shipped configuration
    try:
        mm_b, st_b, av_b, pp_b = [int(v) for v in
                                  os.environ.get("KBUFS", "").split(",")]
    except ValueError:
        mm_b, st_b, av_b, pp_b = 2, 2, 2, 3
    # sim-only: model the HW concurrency of the paired K=64 score matmuls
    # (different PE row groups) by emitting a single matmul per pair.
    # Numerics are wrong, so it is only honored together with the sim-only
    # _no_cc mode - a real build can never pick it up from the environment.
    _sim_pair = bool(os.environ.get("KSIM_PAIR")) and bool(_no_cc)
    nc = bacc.Bacc("TRN2", target_bir_lowering=False, debug=False, num_devices=8)
    BF = mybir.dt.bfloat16
    F32 = mybir.dt.float32
    EXP = mybir.ActivationFunctionType.Exp

    # all inputs host-staged to per-partition-contiguous SBUF layouts so DMA
    # descriptor counts stay low (SEQ dispatch cost ~ descriptors)
    xS = nc.declare_dram_parameter("xS", [NSC, 128, NKD, QC], BF, isOutput=False)
    wqS = nc.declare_dram_parameter("wqS", [128, NKD, 512], BF, isOutput=False)
    wkS = nc.declare_dram_parameter("wkS", [128, NKD, 128], BF, isOutput=False)
    wvS = nc.declare_dram_parameter("wvS", [128, NKD, 128], BF, isOutput=False)
    woS = nc.declare_dram_parameter("woS", [128, NKD, 512], BF, isOutput=False)
    cosS = nc.declare_dram_parameter("cosS", [128, S], F32, isOutput=False)
    sinS = nc.declare_dram_parameter("sinS", [128, S], F32, isOutput=False)
    mask = nc.declare_dram_parameter("mask", [128, 2, QC], BF, isOutput=False)
    out = nc.declare_dram_parameter("out", [512, S], F32, isOutput=True)

    with tile.TileContext(nc) as tc:
        with (
            tc.tile_pool(name="wpool", bufs=1) as wpool,
            tc.tile_pool(name="pers", bufs=1) as pers,
            tc.tile_pool(name="dram", bufs=1, space="DRAM") as dram,
            tc.tile_pool(name="xpool", bufs=12) as xpool,
            tc.tile_pool(name="rtmp", bufs=2) as rtmp,
            tc.tile_pool(name="ppool", bufs=pp_b) as ppool,
            tc.tile_pool(name="npool", bufs=2) as npool,
            tc.tile_pool(name="apool", bufs=2) as apool,
            tc.tile_pool(name="agp", bufs=4) as agp,
            tc.tile_pool(name="opool", bufs=2) as opool,
            tc.tile_pool(name="gps", bufs=mm_b, space="PSUM") as gps,
            tc.tile_pool(name="stps", bufs=st_b, space="PSUM") as stps,
            tc.tile_pool(name="avps", bufs=av_b, space="PSUM") as avps,
        ):
            # ---- persistent weights / constants (one 3D DMA each) ----
            wq_sb = [wpool.tile([128, NKD // 4, 512], BF, name=f"wq_sb{h}",
                                tag=f"wq_sb{h}") for h in range(4)]
            wk_sb = wpool.tile([128, NKD, 128], BF, name="wk_sb", tag="wk_sb")
            wv_sb = wpool.tile([128, NKD, 128], BF, name="wv_sb", tag="wv_sb")
            wo_sb = wpool.tile([128, NKD, 512], BF, name="wo_sb", tag="wo_sb")
            cos_sb = wpool.tile([128, S], F32, name="cos_sb", tag="cos_sb")
            sin_sb = wpool.tile([128, S], F32, name="sin_sb", tag="sin_sb")
            mask_sb = wpool.tile([128, 2, QC], BF, name="mask_sb", tag="mask_sb")

            # ---- persistent activations ----
            qT = [[pers.tile([128, QC], BF, name=f"qT_{rt}_{sc}", tag=f"qT_{rt}_{sc}")
                   for sc in range(NSC)] for rt in range(4)]
            kdup = [[pers.tile([128, QC], BF, name=f"kd_{j}_{sc}", tag=f"kd_{j}_{sc}")
                     for sc in range(NSC)] for j in range(NKVL)]
            vaug = [pers.tile([128, 2, 65], BF, name=f"va_{kt}", tag=f"va_{kt}")
                    for kt in range(NKT)]
            # attention/AG/wo sub-chunks: (qT chunk, col offset, width).
            # The last 512 chunk is split 384+128 so the final serial
            # AG -> gather-load -> wo chain only carries 128 columns.
            SUBS = [(0, 0, QC), (1, 0, QC), (2, 0, QC), (3, 0, 384), (3, 384, 128)]
            # AG buffers carry (128, rt, c) blocks per rank so the gather
            # readback has 4KB-contiguous runs per partition
            ag_in = [dram.tile([128, 4, sub[2]], BF, name=f"ag_in_{i}")
                     for i, sub in enumerate(SUBS)]
            ag_out = [dram.tile([512, 4, sub[2]], BF, name=f"ag_out_{i}")
                      for i, sub in enumerate(SUBS)]

            # hoist x loads so later SP-queue DMAs (which wait on the
            # collectives) never block them; the last chunk is prefetched
            # during qkv(2), still ahead of any AG-dependent DMA in SP order.
            # wq / x chunk 0 are loaded in halves so the first matmuls start
            # after ~1MB of DMA instead of 4MB.
            xts = {}

            def load_x(sc):
                parts = []
                for h in range(4):
                    xt = xpool.tile([128, NKD // 4, QC], BF, name="xt", tag="xt")
                    nc.sync.dma_start(xt[:], xS[sc, :, h * 4:(h + 1) * 4, :])
                    parts.append(xt)
                xts[sc] = parts

            # interleave wq / x quarters so the kd-accumulation of the very
            # first row tile never waits on a distant load
            nc.sync.dma_start(wq_sb[0][:], wqS[:, 0:4, :])
            xts[0] = []
            for h in range(4):
                xt = xpool.tile([128, NKD // 4, QC], BF, name="xt", tag="xt")
                nc.sync.dma_start(xt[:], xS[0, :, h * 4:(h + 1) * 4, :])
                xts[0].append(xt)
                if h < 3:
                    nc.sync.dma_start(wq_sb[h + 1][:],
                                      wqS[:, (h + 1) * 4:(h + 2) * 4, :])
            nc.sync.dma_start(cos_sb[:, 0:QC], cosS[:, 0:QC])
            nc.sync.dma_start(sin_sb[:, 0:QC], sinS[:, 0:QC])
            nc.sync.dma_start(wk_sb[:], wkS[:])
            nc.sync.dma_start(wv_sb[:], wvS[:])
            nc.sync.dma_start(mask_sb[:], mask[:])
            for sc in range(1, NSC - 1):
                load_x(sc)
                nc.sync.dma_start(cos_sb[:, sc * QC:(sc + 1) * QC],
                                  cosS[:, sc * QC:(sc + 1) * QC])
                nc.sync.dma_start(sin_sb[:, sc * QC:(sc + 1) * QC],
                                  sinS[:, sc * QC:(sc + 1) * QC])
            nc.sync.dma_start(cos_sb[:, 3 * QC:], cosS[:, 3 * QC:])
            nc.sync.dma_start(sin_sb[:, 3 * QC:], sinS[:, 3 * QC:])
            nc.sync.dma_start(wo_sb[:], woS[:])

            def qkv_row(sc, rt):
                xt = xts[sc]
                cslice = cos_sb[:, sc * QC:(sc + 1) * QC]
                sslice = sin_sb[:, sc * QC:(sc + 1) * QC]
                if True:  # 0..3: q row tiles; 4: k row tile
                    ps = gps.tile([128, QC], F32, name="gp", tag="gp")
                    for kd in range(NKD):
                        lhsT = (wq_sb[kd // 4][:, kd % 4, rt * 128:(rt + 1) * 128]
                                if rt < 4 else wk_sb[:, kd, :])
                        nc.tensor.matmul(ps[:], lhsT, xt[kd // 4][:, kd % 4, :],
                                         start=(kd == 0), stop=(kd == NKD - 1))
                    # rope in fp32 (bf16 only at the final q/k write):
                    # out = raw*cos + swap32(raw)*sin_signed
                    raw = rtmp.tile([128, QC], F32, name="raw", tag="raw")
                    nc.scalar.copy(raw[:], ps[:])
                    rot = rtmp.tile([128, QC], F32, name="rot", tag="rot")
                    for b32 in range(4):
                        src = (b32 ^ 1) * 32
                        nc.gpsimd.tensor_copy(rot[b32 * 32:(b32 + 1) * 32, :],
                                              raw[src:src + 32, :])
                    t1 = rtmp.tile([128, QC], F32, name="t1", tag="t1")
                    nc.vector.tensor_mul(t1[:], raw[:], cslice)
                    nc.vector.tensor_mul(rot[:], rot[:], sslice)
                    if rt < 4:
                        nc.vector.tensor_add(qT[rt][sc][:], t1[:], rot[:])
                    else:
                        kr = rtmp.tile([128, QC], BF, name="kr", tag="kr")
                        nc.vector.tensor_add(kr[:], t1[:], rot[:])
                        for j in range(NKVL):
                            src = kr[j * 64:(j + 1) * 64, :]
                            nc.gpsimd.tensor_copy(kdup[j][sc][0:64, :], src)
                            nc.gpsimd.tensor_copy(kdup[j][sc][64:128, :], src)
            def v_block(sc):
                # V computed directly in natural (seq, dim) orientation:
                # lhsT = x.T seq-slice, rhs = wv.T -> out (seq, 2*64) + ones col
                xt = xts[sc]
                for tt in range(4):
                    kt = sc * 4 + tt
                    vp = gps.tile([128, QC], F32, name="gp", tag="gp")
                    for kd in range(NKD):
                        nc.tensor.matmul(vp[:, 0:128],
                                         xt[kd // 4][:, kd % 4, tt * 128:(tt + 1) * 128],
                                         wv_sb[:, kd, :],
                                         start=(kd == 0), stop=(kd == NKD - 1))
                    for j in range(NKVL):
                        nc.vector.tensor_copy(vaug[kt][:, j, 0:64],
                                              vp[:, j * 64:(j + 1) * 64])
                        nc.gpsimd.memset(vaug[kt][:, j, 64:65], 1.0)

            def attn_phase(i):
                qc, q0, qn = SUBS[i]
                gqs = qc * QC + q0          # 128-aligned global q start
                t0 = gqs // 128             # first diagonal key tile
                nkt = (gqs + qn) // 128     # causal: key tiles up to sub end
                # attention outputs staged in one tile: (128, rt, seq-chunk)
                atile = apool.tile([128, 4, QC], BF, name="atile", tag="atile")
                for rt in range(4):  # head pair (2rt, 2rt+1); shared kv head
                    j = rt // 2
                    avs = [avps.tile([65, QC], F32, name="av", tag="av")
                           for _ in range(2)]
                    for kt in range(nkt):
                        kb = (kt % 4) * 128
                        # diagonal k-tiles only need q columns >= 128*m
                        # (everything left of that is strictly above the
                        # causal diagonal); qo is the q-column offset
                        m = kt - t0
                        qo = 128 * m if m > 0 else 0
                        n = qn - qo
                        # both halves' scores land in one double-bank PSUM
                        # tile so a single wide exp amortizes the ACT
                        # per-instruction overhead
                        st = stps.tile([128, 2, QC], F32, name="st", tag="st")
                        for half in range(1 if _sim_pair else 2):
                            # operands at partition base 64*half -> the two
                            # K=64 matmuls run in different PE row groups
                            lo, hi = half * 64, half * 64 + 64
                            nc.tensor.matmul(st[:, half, 0:n],
                                             kdup[j][kt // 4][lo:hi, kb:kb + 128],
                                             qT[rt][qc][lo:hi, q0 + qo:q0 + qn],
                                             start=True, stop=True)
                        p = ppool.tile([128, 2, QC], BF, name="p", tag="p")
                        nc.scalar.activation(p[:, :, 0:n], st[:, :, 0:n], EXP,
                                             scale=SCALE)
                        if m >= 0:  # diagonal tile -> triangular 0/1 mask
                            nc.vector.tensor_mul(p[:, :, 0:n], p[:, :, 0:n],
                                                 mask_sb[:, 0:2, 0:n])
                        for half in range(2):
                            nc.tensor.matmul(avs[half][:, qo:qn],
                                             vaug[kt][:, j, :],
                                             p[:, half, 0:n],
                                             start=(kt == 0), stop=(kt == nkt - 1))
                    for half in range(2):
                        av = avs[half]
                        recip = npool.tile([1, QC], F32, name="recip", tag="recip")
                        nc.vector.reciprocal(recip[:, 0:qn], av[64:65, 0:qn])
                        rb = npool.tile([64, QC], F32, name="rb", tag="rb")
                        nc.gpsimd.partition_broadcast(rb[:, 0:qn], recip[:, 0:qn])
                        nc.vector.tensor_mul(
                            atile[half * 64:(half + 1) * 64, rt, 0:qn],
                            av[0:64, 0:qn], rb[:, 0:qn])
                nc.sync.dma_start(ag_in[i][:], atile[:, :, 0:qn])
                if _no_cc:
                    # sim-only mode: local copy instead of the collective, to
                    # measure compute-schedule quality without the cost
                    # model's (pessimistic) collective pricing. NO_CC=2 makes
                    # the gather entirely free (lower bracket).
                    nreps = 1 if str(_no_cc) == "2" else 4
                    for r in range(nreps):
                        nc.gpsimd.dma_start(
                            ag_out[i][r * 128:(r + 1) * 128, :, :], ag_in[i][:])
                else:
                    nc.gpsimd.collective_compute(
                        "AllGather", mybir.AluOpType.bypass,
                        replica_groups=[[0, 1, 2, 3], [4, 5, 6, 7]],
                        ins=[ag_in[i].opt()], outs=[ag_out[i].opt()])

            agts = {}

            def wo_load(i):
                qn = SUBS[i][2]
                # (4 ranks * 128p, rt, c) -> (p, rank, rt, c); kd = rank*4+rt
                agr = ag_out[i].rearrange("(r p) k c -> p r k c", p=128)
                agt = []
                for h in range(2):
                    t = agp.tile([128, 2, 4, qn], BF, name="agt", tag="agt")
                    nc.sync.dma_start(t[:], agr[:, 2 * h:2 * h + 2, :, :])
                    agt.append(t)
                agts[i] = agt

            def wo_block(i, oc):
                qc, q0, qn = SUBS[i]
                col0 = qc * QC + q0
                agt = agts[i]
                if True:
                    ps = gps.tile([128, QC], F32, name="gp", tag="gp")
                    for kd in range(NKD):
                        nc.tensor.matmul(ps[:, 0:qn],
                                         wo_sb[:, kd, oc * 128:(oc + 1) * 128],
                                         agt[kd // 8][:, (kd % 8) // 4, kd % 4, :],
                                         start=(kd == 0), stop=(kd == NKD - 1))
                    ot = opool.tile([128, QC], F32, name="ot", tag="ot")
                    nc.scalar.copy(ot[:, 0:qn], ps[:, 0:qn])
                    nc.sync.dma_start(out[oc * 128:(oc + 1) * 128,
                                          col0:col0 + qn], ot[:, 0:qn])

            def qkv_units(sc):
                units = []
                if sc == 2:
                    units.append(lambda: load_x(3))
                units += [lambda rt=rt: qkv_row(sc, rt) for rt in range(5)]
                units.append(lambda: v_block(sc))
                return units

            def wo_units(i):
                units = [lambda: wo_load(i)]
                units += [lambda oc=oc: wo_block(i, oc) for oc in range(4)]
                return units

            for i, (qc, q0, qn) in enumerate(SUBS):
                if q0 == 0:
                    for u in qkv_units(qc):
                        u()
                attn_phase(i)
                if i >= 2:
                    for u in wo_units(i - 2):
                        u()
                if i == len(SUBS) - 1:
                    for u in wo_units(i - 1):
                        u()
            for u in wo_units(len(SUBS) - 1):
                u()

    nc.compile()
    return nc


def _get_nc():
    global _NC
    if _NC is None:
        _NC = _build()
    return _NC


def _prepare_in_maps(x, freqs_cis, wqkv, wo):
    x = np.asarray(x)
    freqs_cis = np.asarray(freqs_cis)
    wqkv = np.asarray(wqkv)
    wo = np.asarray(wo)

    perm = np.concatenate([np.arange(0, HD, 2), np.arange(1, HD, 2)])
    cos = np.ascontiguousarray(freqs_cis[:, :, 0].T)  # (32, S)
    sin = np.ascontiguousarray(freqs_cis[:, :, 1].T)
    cosS = np.ascontiguousarray(np.concatenate([cos, cos, cos, cos], axis=0),
                                dtype=np.float32)
    sinS = np.ascontiguousarray(np.concatenate([-sin, sin, -sin, sin], axis=0),
                                dtype=np.float32)
    p_i = np.arange(128)[:, None]
    f_i = np.arange(QC)[None, :]
    tri = (f_i >= p_i)
    mask = np.stack([tri, tri], axis=1).astype(BF16)

    def stage(wt):
        # (D, C) with D = 16*128 -> (128, 16, C), per-partition contiguous
        return np.ascontiguousarray(
            wt.reshape(NKD, 128, wt.shape[1]).transpose(1, 0, 2)).astype(BF16)

    xSs = []
    for b in range(2):
        xt = x[b].T  # (D, S)
        xs = xt.reshape(NKD, 128, NSC, QC).transpose(2, 1, 0, 3)
        xSs.append(np.ascontiguousarray(xs).astype(BF16))

    in_maps = []
    for c in range(8):
        b, G = c // 4, c % 4
        qrows = np.concatenate([(8 * G + h) * HD + perm for h in range(NQL)])
        krows = np.concatenate([D + (2 * G + j) * HD + perm for j in range(NKVL)])
        vrows = np.concatenate([D + 512 + (2 * G + j) * HD + np.arange(HD)
                                for j in range(NKVL)])
        in_maps.append({
            "xS": xSs[b],
            "wqS": stage(wqkv[qrows, :].T),
            "wkS": stage(wqkv[krows, :].T),
            "wvS": stage(wqkv[vrows, :].T),
            "woS": stage(wo[512 * G:512 * (G + 1), :].T),
            "cosS": cosS,
            "sinS": sinS,
            "mask": mask,
        })
    return in_maps


def kernel(x, freqs_cis, wqkv, wo, _trace=False):
    in_maps = _prepare_in_maps(x, freqs_cis, wqkv, wo)
    res = run_bass_kernel_spmd(_get_nc(), in_maps, core_ids=list(range(8)),
                               trace=_trace)

    outf = np.empty((2, S, D), np.float32)
    for c in range(8):
        b, G = c // 4, c % 4
        outf[b, :, 512 * G:512 * (G + 1)] = res.results[c]["out"].T
    if _trace:
        kernel.last_exec_time_ns = res.exec_time_ns
        kernel.last_results = res
    return outf



# revision 12
# speedup vs baseline: 1.2135x; 1.2135x over previous
"""Distributed Trainium2 kernel for nn_Attention_68719477187.

RoPE + causal GQA attention (B=2, S=2048, DIM=2048, 32 q heads / 8 kv heads,
head_dim 64) on 8 NeuronCores: TP=8 over heads.

Per core c: 4 q heads {4c..4c+3} (2 pair-tiles rt) + 1 kv head (c), BOTH
batches.  Output is token-sharded: after attention, one 8-core AllToAll per
sequence sub-chunk redistributes attention outputs so core r owns
(batch r//4, a token slice), then each core runs the full wo on its tokens.
AllToAll moves 4x fewer bytes than an AllGather scheme (0.5MB/round vs 2MB).

  1. qkv: 3 row-tiles per (batch, chunk): 2x q-pairs + packed [k;v] tile.
     RoPE applied in transposed layout (64-dim pre-permuted evens|odds).
     v rows leave the packed tile via DMA-transpose into token-major vaug
     (ones column appended so the softmax denominator falls out of the AV
     matmul).
  2. scores.T tiles (keys on partitions) -> exp (no max subtraction; scores
     are O(5) so fp32 exp is safe) -> causal 0/1 mask on the diagonal 128
     cols -> AV matmul against vaug.
  3. Per sub-chunk: atile (attn.T, bf16) staged to DRAM, 8-core AllToAll,
     gather to [128p, 16kd, tok] and token-major wo: lhsT = attn.T block
     (stationary), rhs = wo.T slice (moving, 512 wide), PSUM accum over kd.
     Subs 3+4 share one wo chain (their gathers land in one tile) so the
     tail after the last attention is just a 128KB AllToAll + one wo chain.

Compute in bf16 (fp32 PSUM accumulation), output fp32.
"""

import sys

if "/opt/trn_rl_repo" not in sys.path:
    sys.path.insert(0, "/opt/trn_rl_repo")

import numpy as np
import ml_dtypes

from concourse import bacc, tile, mybir
from concourse.bass_utils import run_bass_kernel_spmd

BF16 = ml_dtypes.bfloat16

S = 2048          # sequence length
D = 2048          # model dim
HD = 64           # head dim
NB = 2            # batches (every core sees both)
QC = 512          # q chunk (matmul free dim)
NSC = S // QC     # 4 seq chunks
NKD = D // 128    # 16 contraction tiles
NKT = S // 128    # 16 key tiles
SCALE = HD ** -0.5

# attention/AllToAll/wo sub-chunks: (qT chunk, col offset, width).  The last
# 512 chunk is split 384+128 so the final serial a2a -> gather -> wo chain
# carries only 128KB; subs 3+4 share one wo chain (128 tokens/core).
SUBS = [(0, 0, QC), (1, 0, QC), (2, 0, QC), (3, 0, 384), (3, 384, 128)]
SUB_T4 = [qn // 4 for _, _, qn in SUBS]           # tokens/core per sub
SUB_BASE = [qc * QC + q0 for qc, q0, _ in SUBS]   # global token base
SUB_LOCAL = [0, 128, 256, 384, 480]               # local out row base

_NC = None


def _build():
    nc = bacc.Bacc("TRN2", target_bir_lowering=False, debug=False, num_devices=8)
    BF = mybir.dt.bfloat16
    F32 = mybir.dt.float32
    EXP = mybir.ActivationFunctionType.Exp

    # host-staged, per-partition-contiguous layouts (low descriptor counts)
    xS = nc.declare_dram_parameter("xS", [NB, NSC, 128, NKD, QC], BF, isOutput=False)
    wqS = nc.declare_dram_parameter("wqS", [128, NKD, 256], BF, isOutput=False)
    wkvS = nc.declare_dram_parameter("wkvS", [128, NKD, 128], BF, isOutput=False)
    woTS = nc.declare_dram_parameter("woTS", [128, NKD, D], BF, isOutput=False)
    cosS = nc.declare_dram_parameter("cosS", [128, S], F32, isOutput=False)
    sinS = nc.declare_dram_parameter("sinS", [128, S], F32, isOutput=False)
    mask = nc.declare_dram_parameter("mask", [128, 2, 128], BF, isOutput=False)
    out = nc.declare_dram_parameter("out", [512, D], F32, isOutput=True)

    with tile.TileContext(nc) as tc:
        with (
            tc.tile_pool(name="wpool", bufs=1) as wpool,
            tc.tile_pool(name="pers", bufs=1) as pers,
            tc.tile_pool(name="dram", bufs=1, space="DRAM") as dram,
            tc.tile_pool(name="xpool", bufs=10) as xpool,
            tc.tile_pool(name="cspool", bufs=2) as cspool,
            tc.tile_pool(name="rtmp", bufs=2) as rtmp,
            tc.tile_pool(name="vtp", bufs=2) as vtp,
            tc.tile_pool(name="ppool", bufs=3) as ppool,
            tc.tile_pool(name="npool", bufs=2) as npool,
            tc.tile_pool(name="apool", bufs=4) as apool,
            tc.tile_pool(name="agp", bufs=2) as agp,
            tc.tile_pool(name="opool", bufs=2) as opool,
            tc.tile_pool(name="gps", bufs=2, space="PSUM") as gps,
            tc.tile_pool(name="stps", bufs=2, space="PSUM") as stps,
            tc.tile_pool(name="avps", bufs=2, space="PSUM") as avps,
        ):
            # ---- persistent weights / constants ----
            wq_sb = [wpool.tile([128, NKD // 4, 256], BF, name=f"wq_sb{h}",
                                tag=f"wq_sb{h}") for h in range(4)]
            wkv_sb = wpool.tile([128, NKD, 128], BF, name="wkv_sb", tag="wkv_sb")
            woT_sb = wpool.tile([128, NKD, D], BF, name="woT_sb", tag="woT_sb")
            mask_sb = wpool.tile([128, 2, 128], BF, name="mask_sb", tag="mask_sb")

            # ---- persistent activations ----
            qT = [[[pers.tile([128, QC], BF, name=f"qT_{rt}_{b}_{sc}",
                              tag=f"qT_{rt}_{b}_{sc}") for sc in range(NSC)]
                   for b in range(NB)] for rt in range(2)]
            kdup = [[pers.tile([128, QC], BF, name=f"kd_{b}_{sc}",
                               tag=f"kd_{b}_{sc}") for sc in range(NSC)]
                    for b in range(NB)]
            vaug = [[pers.tile([128, 65], BF, name=f"va_{b}_{kt}",
                               tag=f"va_{b}_{kt}") for kt in range(NKT)]
                    for b in range(NB)]
            for b in range(NB):
                for kt in range(NKT):
                    nc.gpsimd.memset(vaug[b][kt][:, 64:65], 1.0)

            # AllToAll buffers: [8 dest/src ranks, 2rt, 128p, T4 tokens]
            # (rt before p so the gather's (src, rt) dims merge into one run)
            a2a_in = [dram.tile([8, 2, 128, SUB_T4[i]], BF, name=f"a2a_in_{i}")
                      for i in range(len(SUBS))]
            a2a_out = [dram.tile([8, 2, 128, SUB_T4[i]], BF, name=f"a2a_out_{i}")
                       for i in range(len(SUBS))]

            # ---- hoisted loads.  SP queue: loads + v-transposes + gathers;
            # ACT queue: a2a staging + out writes (never wait on collectives
            # before compute-dependent DMAs in FIFO order).
            xts = {}
            cs_tiles = {}

            def load_x(b, sc):
                parts = []
                for h in range(4):
                    xt = xpool.tile([128, NKD // 4, QC], BF, name="xt", tag="xt")
                    nc.sync.dma_start(xt[:], xS[b, sc, :, h * 4:(h + 1) * 4, :])
                    parts.append(xt)
                xts[(b, sc)] = parts

            def load_cs(sc):
                ct = cspool.tile([128, QC], F32, name="cosc", tag="cosc")
                st = cspool.tile([128, QC], F32, name="sinc", tag="sinc")
                nc.sync.dma_start(ct[:], cosS[:, sc * QC:(sc + 1) * QC])
                nc.sync.dma_start(st[:], sinS[:, sc * QC:(sc + 1) * QC])
                cs_tiles[sc] = (ct, st)

            def load_wo(h):
                nc.sync.dma_start(woT_sb[:, h * 4:(h + 1) * 4, :],
                                  woTS[:, h * 4:(h + 1) * 4, :])

            # interleave wq / x(b0,0) quarters so the first matmul starts
            # after ~1MB of DMA
            nc.sync.dma_start(wq_sb[0][:], wqS[:, 0:4, :])
            xts[(0, 0)] = []
            for h in range(4):
                xt = xpool.tile([128, NKD // 4, QC], BF, name="xt", tag="xt")
                nc.sync.dma_start(xt[:], xS[0, 0, :, h * 4:(h + 1) * 4, :])
                xts[(0, 0)].append(xt)
                if h < 3:
                    nc.sync.dma_start(wq_sb[h + 1][:],
                                      wqS[:, (h + 1) * 4:(h + 2) * 4, :])
            load_cs(0)
            nc.sync.dma_start(wkv_sb[:], wkvS[:])
            nc.sync.dma_start(mask_sb[:], mask[:])
            load_x(1, 0)
            load_x(0, 1)

            PENDING = [
                lambda: load_x(1, 1),
                lambda: load_cs(1),
                lambda: load_x(0, 2),
                lambda: load_wo(0),
                lambda: load_x(1, 2),
                lambda: load_cs(2),
                lambda: load_x(0, 3),
                lambda: load_wo(1),
                lambda: load_x(1, 3),
                lambda: load_cs(3),
                lambda: load_wo(2),
                lambda: load_wo(3),
            ]

            def consume_pending(k):
                for _ in range(k):
                    if PENDING:
                        PENDING.pop(0)()

            def rope(ps, sc, dst, nr):
                """nr-row rope: dst = raw*cos + swap32(raw)*sin_signed."""
                ct, sn = cs_tiles[sc]
                raw = rtmp.tile([128, QC], F32, name="raw", tag="raw")
                nc.scalar.copy(raw[0:nr, :], ps[0:nr, :])
                rot = rtmp.tile([128, QC], F32, name="rot", tag="rot")
                for b32 in range(nr // 32):
                    src = (b32 ^ 1) * 32
                    nc.gpsimd.tensor_copy(rot[b32 * 32:(b32 + 1) * 32, :],
                                          raw[src:src + 32, :])
                t1 = rtmp.tile([128, QC], F32, name="t1", tag="t1")
                nc.vector.tensor_mul(t1[0:nr, :], raw[0:nr, :], ct[0:nr, :])
                nc.vector.tensor_mul(rot[0:nr, :], rot[0:nr, :], sn[0:nr, :])
                nc.vector.tensor_add(dst[0:nr, :], t1[0:nr, :], rot[0:nr, :])

            def qkv_block(b, sc):
                xt = xts[(b, sc)]
                for rt in range(2):
                    ps = gps.tile([128, QC], F32, name="gp", tag="gp")
                    for kd in range(NKD):
                        nc.tensor.matmul(
                            ps[:], wq_sb[kd // 4][:, kd % 4, rt * 128:(rt + 1) * 128],
                            xt[kd // 4][:, kd % 4, :],
                            start=(kd == 0), stop=(kd == NKD - 1))
                    rope(ps, sc, qT[rt][b][sc], 128)
                # packed [k(64, perm'd); v(64, natural)] tile
                ps = gps.tile([128, QC], F32, name="gp", tag="gp")
                for kd in range(NKD):
                    nc.tensor.matmul(ps[:], wkv_sb[:, kd, :],
                                     xt[kd // 4][:, kd % 4, :],
                                     start=(kd == 0), stop=(kd == NKD - 1))
                # k rows 0:64 -> rope -> duplicate into both kdup halves
                kr = rtmp.tile([64, QC], BF, name="kr", tag="kr")
                rope(ps, sc, kr, 64)
                nc.gpsimd.tensor_copy(kdup[b][sc][0:64, :], kr[:])
                nc.gpsimd.tensor_copy(kdup[b][sc][64:128, :], kr[:])
                # v rows 64:128 -> bf16 -> DMA-transpose into token-major vaug
                vtmp = vtp.tile([64, QC], BF, name="vtmp", tag="vtmp")
                nc.scalar.copy(vtmp[:], ps[64:128, :])
                for tt in range(4):
                    kt = sc * 4 + tt
                    nc.sync.dma_start_transpose(
                        vaug[b][kt][:, 0:64], vtmp[:, tt * 128:(tt + 1) * 128])

            def attn_phase(i):
                qc, q0, qn = SUBS[i]
                gqs = qc * QC + q0          # 128-aligned global q start
                t0 = gqs // 128             # first diagonal key tile
                nkt = (gqs + qn) // 128     # causal: key tiles up to sub end
                for b in range(NB):
                    atile = apool.tile([128, 2, QC], BF, name="atile", tag="atile")
                    for rt in range(2):
                        avs = [avps.tile([65, QC], F32, name="av", tag="av")
                               for _ in range(2)]
                        for kt in range(nkt):
                            kb = (kt % 4) * 128
                            m = kt - t0
                            qo = 128 * m if m > 0 else 0
                            n = qn - qo
                            st = stps.tile([128, 2, QC], F32, name="st", tag="st")
                            for half in range(2):
                                lo, hi = half * 64, half * 64 + 64
                                nc.tensor.matmul(
                                    st[:, half, 0:n],
                                    kdup[b][kt // 4][lo:hi, kb:kb + 128],
                                    qT[rt][b][qc][lo:hi, q0 + qo:q0 + qn],
                                    start=True, stop=True)
                            p = ppool.tile([128, 2, QC], BF, name="p", tag="p")
                            nc.scalar.activation(p[:, :, 0:n], st[:, :, 0:n],
                                                 EXP, scale=SCALE)
                            if m >= 0:  # diagonal block: triangular 0/1 mask
                                nc.vector.tensor_mul(p[:, :, 0:128],
                                                     p[:, :, 0:128], mask_sb[:])
                            for half in range(2):
                                nc.tensor.matmul(avs[half][:, qo:qn],
                                                 vaug[b][kt][:, :],
                                                 p[:, half, 0:n],
                                                 start=(kt == 0),
                                                 stop=(kt == nkt - 1))
                        for half in range(2):
                            av = avs[half]
                            recip = npool.tile([1, QC], F32, name="recip",
                                               tag="recip")
                            nc.vector.reciprocal(recip[:, 0:qn], av[64:65, 0:qn])
                            rb = npool.tile([64, QC], F32, name="rb", tag="rb")
                            nc.gpsimd.partition_broadcast(rb[:, 0:qn],
                                                          recip[:, 0:qn])
                            nc.vector.tensor_mul(
                                atile[half * 64:(half + 1) * 64, rt, 0:qn],
                                av[0:64, 0:qn], rb[:, 0:qn])
                    # stage to DRAM (ACT queue): dest 4b+d gets token block d
                    for r in range(2):
                        nc.scalar.dma_start(
                            a2a_in[i][4 * b:4 * b + 4, r, :, :]
                            .rearrange("d p t -> p d t"),
                            atile[:, r, 0:qn].rearrange("p (d t) -> p d t",
                                                        d=4))
                nc.gpsimd.collective_compute(
                    "AllToAll", mybir.AluOpType.bypass,
                    replica_groups=[[0, 1, 2, 3, 4, 5, 6, 7]],
                    ins=[a2a_in[i].opt()], outs=[a2a_out[i].opt()])

            agts = {}

            def gather(i, agt=None, col0=0):
                t4 = SUB_T4[i]
                if agt is None:
                    agt = agp.tile([128, NKD, 128], BF, name="agt", tag="agt")
                nc.sync.dma_start(
                    agt[:, :, col0:col0 + t4],
                    a2a_out[i].rearrange("c r p t -> p (c r) t"))
                agts[i] = agt
                return agt

            def wo_chain(i, ntok):
                """token-major wo for sub i's gathered tokens (<=128)."""
                agt = agts[i]
                for oh in range(2):
                    ot = opool.tile([128, D // 2], F32, name="ot", tag="ot")
                    for oc in range(2):
                        ps = gps.tile([128, QC], F32, name="gp", tag="gp")
                        for kd in range(NKD):
                            nc.tensor.matmul(
                                ps[0:ntok, :], agt[:, kd, 0:ntok],
                                woT_sb[:, kd, (2 * oh + oc) * QC:
                                       (2 * oh + oc + 1) * QC],
                                start=(kd == 0), stop=(kd == NKD - 1))
                        nc.scalar.copy(ot[0:ntok, oc * QC:(oc + 1) * QC],
                                       ps[0:ntok, :])
                    nc.scalar.dma_start(
                        out[SUB_LOCAL[i]:SUB_LOCAL[i] + ntok,
                            oh * (D // 2):(oh + 1) * (D // 2)],
                        ot[0:ntok, :])

            # ---- schedule ----
            # i=0: qkv(0) attn(0) | i=1: qkv(1) attn(1) | i=2: qkv(2) attn(2)
            # wo(0) | i=3: qkv(3) attn(3) wo(1) wo(2) | i=4: attn(4) wo(3+4)
            for i, (qc, q0, qn) in enumerate(SUBS):
                if q0 == 0:
                    qkv_block(0, qc)
                    consume_pending(1)
                    qkv_block(1, qc)
                    consume_pending(3)
                attn_phase(i)
                if i == 2:
                    gather(0)
                    wo_chain(0, 128)
                if i == 3:
                    gather(1)
                    wo_chain(1, 128)
                    gather(2)
                    wo_chain(2, 128)
            # tail: subs 3+4 share one agt tile and one wo chain
            agt34 = gather(3, col0=0)
            gather(4, agt=agt34, col0=96)
            agts[3] = agt34
            wo_chain(3, 128)

    nc.compile()
    return nc


def _get_nc():
    global _NC
    if _NC is None:
        _NC = _build()
    return _NC


def _prepare_in_maps(x, freqs_cis, wqkv, wo):
    x = np.asarray(x)
    freqs_cis = np.asarray(freqs_cis)
    wqkv = np.asarray(wqkv)
    wo = np.asarray(wo)

    perm = np.concatenate([np.arange(0, HD, 2), np.arange(1, HD, 2)])
    cos = np.ascontiguousarray(freqs_cis[:, :, 0].T)  # (32, S)
    sin = np.ascontiguousarray(freqs_cis[:, :, 1].T)
    cosS = np.ascontiguousarray(np.concatenate([cos, cos, cos, cos], axis=0),
                                dtype=np.float32)
    sinS = np.ascontiguousarray(np.concatenate([-sin, sin, -sin, sin], axis=0),
                                dtype=np.float32)
    p_i = np.arange(128)[:, None]
    f_i = np.arange(128)[None, :]
    maskA = np.stack([(f_i >= p_i)] * 2, axis=1).astype(BF16)

    def stage(wt):
        # (D, C) with D = 16*128 -> (128, 16, C), per-partition contiguous
        return np.ascontiguousarray(
            wt.reshape(NKD, 128, wt.shape[1]).transpose(1, 0, 2)).astype(BF16)

    xSs = np.empty((NB, NSC, 128, NKD, QC), dtype=BF16)
    for b in range(NB):
        xt = x[b].T  # (D, S)
        xSs[b] = xt.reshape(NKD, 128, NSC, QC).transpose(2, 1, 0, 3)
    xSs = np.ascontiguousarray(xSs)

    # wo.T staged by attention-column order: kd=(src_core, rt), p=half*64+d
    # -> attn col (4*src + 2*rt + half)*64 + d ; identical for every core.
    p_idx = np.arange(128)
    kd_idx = np.arange(NKD)
    cols = ((4 * (kd_idx[None, :] // 2) + 2 * (kd_idx[None, :] % 2)
             + (p_idx[:, None] // 64)) * 64 + (p_idx[:, None] % 64))
    woTS = np.ascontiguousarray(
        wo[:, cols].transpose(1, 2, 0)).astype(BF16)  # [128, 16, 2048]

    in_maps = []
    for c in range(8):
        qrows = np.concatenate([(4 * c + h) * HD + perm for h in range(4)])
        krows = D + c * HD + perm
        vrows = D + 512 + c * HD + np.arange(HD)
        kvrows = np.concatenate([krows, vrows])
        in_maps.append({
            "xS": xSs,
            "wqS": stage(wqkv[qrows, :].T),
            "wkvS": stage(wqkv[kvrows, :].T),
            "woTS": woTS,
            "cosS": cosS,
            "sinS": sinS,
            "mask": maskA,
        })
    return in_maps


def kernel(x, freqs_cis, wqkv, wo, _trace=False):
    in_maps = _prepare_in_maps(x, freqs_cis, wqkv, wo)
    res = run_bass_kernel_spmd(_get_nc(), in_maps, core_ids=list(range(8)),
                               trace=_trace)

    outf = np.empty((2, S, D), np.float32)
    for c in range(8):
        b, blk = c // 4, c % 4
        o = res.results[c]["out"]  # [512, 2048] fp32
        for i in range(len(SUBS)):
            t4 = SUB_T4[i]
            g0 = SUB_BASE[i] + blk * t4
            l0 = SUB_LOCAL[i]
            outf[b, g0:g0 + t4, :] = o[l0:l0 + t4, :]
    if _trace:
        kernel.last_exec_time_ns = res.exec_time_ns
        kernel.last_results = res
    return outf
